# revision 20
# baseline (speedup 1.0000x reference)
"""Trainium2 Bass kernel for BlurGenerationPair.

Reference computation (B=128, T=512, D=128):
    avged_seq[b] = BlurMat[b]^T @ seq[b]          # the only heavy compute
    R[b, t]      = (t+1)/len_seq[b]  for t < len_seq[b] else 0
    avged_R[b,t] = (t+1)/avged_len[b] for t < avged_len[b] else 0
    outputs: (SeqtoBlur, avged_seq, R, avged_R, avged_len)

SeqtoBlur and avged_len are identity pass-throughs and R/avged_R are tiny
host-side ramps, so the device work is the batched ragged matmul.

Strategy: BlurMat[b] is highly structured — all nonzeros live in a
[t_hi, i_hi] top-left rectangle (t_hi ~ len_seq[b], i_hi ~ avged_len[b]),
and ~20% of samples are entirely zero. The host computes each sample's
nonzero bounding box, load-balances the nonzero samples across the 8
cores (data parallel over batch), and packs each core's cropped blocks
directly in the SBUF image layout the matmuls want ([128 partitions x
free], contraction k-tiles side by side). Remainder k-tiles (< 128 rows)
from different samples are stacked vertically in shared column strips at
partition offsets {0, 32, 64} so the transferred rectangles carry almost
no dead bytes. The device program is a handful of large contiguous DMAs
per core plus the matmul pyramid (out.T = seq.T @ blur per sample, PSUM
accumulated over k-tiles). Per-slot shapes are static: the max over the
8 cores at each slot rank. Output rectangles are scattered back into a
zero-filled full-shape array on the host.
"""

import numpy as np

import concourse.bacc as bacc
import concourse.mybir as mybir
from concourse.tile import TileContext
from concourse.bass_utils import run_bass_kernel_spmd

B, T, D = 128, 512, 128
N_CORES = 8
P = 128

# pipelining granularity: chunk the strip sequence so each blur group
# transfer is at least this many bytes
GROUP_BYTES = 512 * 1024

_cache = {}


# ---------------------------------------------------------------- planning

def _bounding_boxes(blur):
    """Per-sample [t_hi, i_hi) nonzero bounding boxes of blur [B,T,T]."""
    nz_rows = np.any(blur != 0.0, axis=2)  # [B, T]
    nz_cols = np.any(blur != 0.0, axis=1)  # [B, T]
    t_hi = np.zeros(B, np.int64)
    i_hi = np.zeros(B, np.int64)
    for b in range(B):
        r = np.nonzero(nz_rows[b])[0]
        c = np.nonzero(nz_cols[b])[0]
        if r.size:
            t_hi[b] = r[-1] + 1
            i_hi[b] = c[-1] + 1
    return t_hi, i_hi


def _arrange_bin(members):
    """HW matmul partition-access rule: base 0 -> span <=128, base 32 ->
    span <=32, base 64 -> span <=64 (base 96 illegal). Place members
    (idx, rows) ascending by rows with a cursor rounded up to the
    smallest legal base. Returns [(idx, rows, offset)] or None."""
    span = {0: P, 32: 32, 64: 64}
    placed = []
    cur = 0
    for ix, r in sorted(members, key=lambda m: m[1]):
        off = None
        for base in (0, 32, 64):
            if base >= cur and r <= span[base] and base + r <= P:
                off = base
                break
        if off is None:
            return None
        placed.append((ix, r, off))
        cur = off + r
    return placed


def _bin_pack_rems(rems):
    """Greedy bin packing of remainder k-tiles (rows, width) into 128-row
    strips under the partition-base access rules.
    Returns list of bins: (width, [(item_idx, rows, offset)])."""
    order = sorted(range(len(rems)), key=lambda x: -rems[x][1])
    bins = []  # (wmax, [(idx, rows)])
    for ix in order:
        r, w = rems[ix]
        best = None
        for bi, (wmax, members) in enumerate(bins):
            if _arrange_bin(members + [(ix, r)]) is None:
                continue
            if any(abs(m[0] - ix) > 4 for m in members):
                continue  # keep members at nearby slot ranks (psum lifetime)
            used = sum(m[1] for m in members)
            c = max(0, w - wmax) * used + max(0, wmax - w) * r
            if best is None or c < best[0]:
                best = (c, bi)
        newc = (P - r) * w
        if best is not None and best[0] <= newc:
            bi = best[1]
            wmax, members = bins[bi]
            bins[bi] = (max(wmax, w), members + [(ix, r)])
        else:
            bins.append((w, [(ix, r)]))
    return [(wmax, _arrange_bin(members)) for wmax, members in bins]


def _grid_cost(grid, t32f, i4f):
    """Proxy for per-core image bytes of an assignment (incl remainder
    stacking)."""
    tot = 0
    rems = []
    for row in grid:
        Ts = [t32f[s] for s in row if s >= 0]
        Is = [i4f[s] for s in row if s >= 0]
        if not Ts:
            continue
        Tj, Ij = max(Ts), max(Is)
        Kf, r = Tj // P, Tj % P
        tot += Kf * Ij * P + Kf * P * P + Ij * P
        if r:
            rems.append((r, Ij))
    for wmax, _members in _bin_pack_rems(rems):
        tot += P * wmax + P * P
    return tot * 4


def _plan(t_hi, i_hi):
    """Assign nonzero samples to (core, slot) cells and build the static
    per-slot shape template (max over cores at each slot rank).
    t is at 32-row grain (partition-offset stacking granularity),
    i at 4-col grain."""
    nz = np.nonzero(t_hi)[0]
    if nz.size == 0:
        return None
    t32 = np.minimum((t_hi[nz] + 31) // 32 * 32, T)
    i4 = np.minimum((i_hi[nz] + 3) // 4 * 4, T)
    cost = t32 * (i4 + P) + i4 * P
    order = np.argsort(-cost, kind="stable")
    S = int(np.ceil(nz.size / N_CORES))
    grid = []
    k = 0
    for j in range(S):
        row = []
        for c in range(N_CORES):
            row.append(nz[order[k]] if k < nz.size else -1)
            k += 1
        if j % 2 == 1:
            row = row[::-1]  # serpentine for per-core cost balance
        grid.append(row)

    t32f = np.zeros(B, np.int64)
    i4f = np.zeros(B, np.int64)
    t32f[nz] = t32
    i4f[nz] = i4

    def tmpl(row):
        Ts = [t32f[s] for s in row if s >= 0]
        Is = [i4f[s] for s in row if s >= 0]
        if not Ts:
            return 0, 0
        return max(Ts), max(Is)

    # local-swap refinement: any pair of slot rows within a core
    for _ in range(6):
        improved = False
        for j1 in range(S):
            for j2 in range(j1 + 1, S):
                for c in range(N_CORES):
                    a, b_ = grid[j1][c], grid[j2][c]
                    if a < 0 or b_ < 0:
                        continue
                    cur = _grid_cost(grid, t32f, i4f)
                    grid[j1][c], grid[j2][c] = b_, a
                    new = _grid_cost(grid, t32f, i4f)
                    if new < cur:
                        improved = True
                    else:
                        grid[j1][c], grid[j2][c] = a, b_
        if not improved:
            break

    template = [tmpl(row) for row in grid]
    keep = [j for j, t in enumerate(template) if t[0] > 0 and t[1] > 0]
    return [grid[j] for j in keep], [template[j] for j in keep]


class _Layout:
    """Static geometry shared by program builder, packer and scatterer.

    Blur/seq images are sequences of column strips:
      - full strip: one slot's k-tile, 128 rows of data
      - bin strip: stacked remainder k-tiles of 1-3 slots at partition
        offsets {0, 32, 64}
    Strips are positioned in slot order (bins at their first member), so a
    slot's dependencies are always in its own or earlier DMA groups.
    """

    def __init__(self, template):
        self.template = template
        S = len(template)
        self.slots = []
        for Tj, Ij in template:
            self.slots.append(
                dict(T=Tj, I=Ij, Kf=Tj // P, r=Tj % P, bo=[], so=[], oo=0)
            )

        # pass 1: full strips in slot order; remainder k-tiles bin-packed
        # into shared strips inserted after their LAST member's full strips.
        # strip: dict(w=blur width, members=[(slot, kind, k, poff, rows)])
        rems = []  # (slot, rows, width), in slot order
        for j, sl in enumerate(self.slots):
            if sl["r"]:
                rems.append((j, sl["r"], sl["I"]))
        bin_after = {}  # slot -> list of bin strip dicts to insert after it
        for wmax, placed in _bin_pack_rems([(r, w) for _j, r, w in rems]):
            members = []
            for ix, r, off in placed:
                j = rems[ix][0]
                members.append((j, "rem", self.slots[j]["Kf"], off, r))
            last = max(m[0] for m in members)
            bin_after.setdefault(last, []).append(
                dict(w=wmax, members=members, bin=True)
            )
        self.strips = []
        for j, sl in enumerate(self.slots):
            Ij = sl["I"]
            for k in range(sl["Kf"]):
                self.strips.append(
                    dict(w=Ij, members=[(j, "full", k, 0, P)], bin=False)
                )
            for st in bin_after.get(j, []):
                self.strips.append(st)

        # pass 2: column offsets; per-slot k-tile -> (strip col, poff, rows)
        bc = sc = oc = 0
        for st in self.strips:
            st["bc"] = bc
            st["sc"] = sc
            for (j, kind, k, poff, rows) in st["members"]:
                self.slots[j]["bo"].append((k, bc, poff, rows))
                self.slots[j]["so"].append((k, sc, poff, rows))
            bc += st["w"]
            sc += D
        for sl in self.slots:
            sl["bo"].sort()
            sl["so"].sort()
            sl["oo"] = oc
            oc += sl["I"]
        self.blur_cols = bc
        self.seq_cols = sc
        self.out_cols = oc

        # groups: consecutive strips, >= GROUP_BYTES of blur each
        self.groups = []  # list of (strip_lo, strip_hi)
        lo = 0
        acc = 0
        for si, st in enumerate(self.strips):
            acc += st["w"] * P * 4
            if acc >= GROUP_BYTES:
                self.groups.append((lo, si + 1))
                lo = si + 1
                acc = 0
        if lo < len(self.strips):
            self.groups.append((lo, len(self.strips)))
        # strip index -> group index
        self.strip_group = {}
        for g, (a, b_) in enumerate(self.groups):
            for si in range(a, b_):
                self.strip_group[si] = g
        # slot -> last group it depends on (for compute emission order)
        slot_last_group = [0] * S
        for si, st in enumerate(self.strips):
            for (j, *_rest) in st["members"]:
                slot_last_group[j] = max(slot_last_group[j], self.strip_group[si])
        self.group_slots = [[] for _ in self.groups]
        for j in range(S):
            self.group_slots[slot_last_group[j]].append(j)


# ---------------------------------------------------------------- program

def _build_program(layout, reps=None):
    """reps: if set, wrap the body in a hardware loop executing it `reps`
    times — used only for steady-state benchmarking (amortizes the ~90ms
    axon dispatch overhead out of wall-clock measurements)."""
    import contextlib

    nc = bacc.Bacc("TRN2")
    f32 = mybir.dt.float32

    blur_t = nc.dram_tensor("blur", [P, layout.blur_cols], f32, kind="ExternalInput")
    seq_t = nc.dram_tensor("seq", [P, layout.seq_cols], f32, kind="ExternalInput")
    out_t = nc.dram_tensor("out", [P, layout.out_cols], f32, kind="ExternalOutput")

    with TileContext(nc) as tc:
        with (
            tc.tile_pool(name="sb", bufs=1) as sb,
            tc.tile_pool(name="psum", bufs=8, space="PSUM") as psum_p,
            tc.For_i(0, reps, 1) if reps else contextlib.nullcontext(),
        ):
            qbytes = [0, 0]

            def q_engine(nbytes):
                qi = 0 if qbytes[0] <= qbytes[1] else 1
                qbytes[qi] += nbytes
                return nc.sync if qi == 0 else nc.scalar

            # SBUF residency: keep all groups resident when they fit in the
            # SBUF budget; otherwise ring-buffer the group tags (Tile then
            # serializes a reused slot's load behind its prior consumers).
            ngroups = len(layout.groups)
            total_cols = layout.blur_cols + layout.seq_cols + min(
                len(layout.slots), 8
            ) * 512
            budget_cols = 160 * 1024 // 4  # ~160KB per partition
            ring = ngroups
            if total_cols > budget_cols:
                gw = []
                for a, b_ in layout.groups:
                    bc1 = layout.blur_cols if b_ == len(layout.strips) else layout.strips[b_]["bc"]
                    gw.append(bc1 - layout.strips[a]["bc"] + (b_ - a) * D)
                wmax = max(gw)
                ring = max(3, (budget_cols - 8 * 512) // max(wmax, 1))

            # per-group SBUF tiles + group column base
            blur_tiles = []
            seq_tiles = []
            gbase = []
            for g, (a, b_) in enumerate(layout.groups):
                bc0 = layout.strips[a]["bc"]
                sc0 = layout.strips[a]["sc"]
                bc1 = layout.blur_cols if b_ == len(layout.strips) else layout.strips[b_]["bc"]
                sc1 = layout.seq_cols if b_ == len(layout.strips) else layout.strips[b_]["sc"]
                gbase.append((bc0, sc0))
                bt = sb.tile([P, bc1 - bc0], f32, tag=f"blur{g % ring}")
                q_engine((bc1 - bc0) * P * 4).dma_start(out=bt[:], in_=blur_t[:, bc0:bc1])
                st = sb.tile([P, sc1 - sc0], f32, tag=f"seq{g % ring}")
                q_engine((sc1 - sc0) * P * 4).dma_start(out=st[:], in_=seq_t[:, sc0:sc1])
                blur_tiles.append(bt)
                seq_tiles.append(st)

                # compute for slots whose last dependency is group g
                for j in layout.group_slots[g]:
                    sl = layout.slots[j]
                    Ij, oo = sl["I"], sl["oo"]
                    nk = len(sl["bo"])
                    ps = psum_p.tile([P, 512], f32, tag="ps")
                    for ki, (k, bc, poff, rows) in enumerate(sl["bo"]):
                        _, sc, _, _ = sl["so"][ki]
                        sg = layout.strip_group[
                            _strip_at(layout, bc)
                        ]
                        bg0, sg0 = gbase[sg]
                        nc.tensor.matmul(
                            ps[:, :Ij],
                            lhsT=seq_tiles[sg][poff : poff + rows, sc - sg0 : sc - sg0 + D],
                            rhs=blur_tiles[sg][poff : poff + rows, bc - bg0 : bc - bg0 + Ij],
                            start=(ki == 0),
                            stop=(ki == nk - 1),
                        )
                    og = sb.tile([P, Ij], f32, tag=f"out{j % 8}")
                    nc.vector.tensor_copy(og[:], ps[:, :Ij])
                    q_engine(Ij * P * 4).dma_start(
                        out=out_t[:, oo : oo + Ij], in_=og[:]
                    )
    nc.finalize()
    return nc


def _strip_at(layout, bc):
    # strips are ordered by bc; binary search
    lo, hi = 0, len(layout.strips)
    while lo + 1 < hi:
        mid = (lo + hi) // 2
        if layout.strips[mid]["bc"] <= bc:
            lo = mid
        else:
            hi = mid
    return lo


# ---------------------------------------------------------------- packing

def _pack_inputs(grid, layout, blur_np, seq_np, t_hi):
    """Build per-core SBUF-image blur/seq buffers [128, cols]."""
    in_maps = []
    for c in range(N_CORES):
        bimg = np.zeros((P, layout.blur_cols), np.float32)
        simg = np.zeros((P, layout.seq_cols), np.float32)
        for j, sl in enumerate(layout.slots):
            s = grid[j][c]
            if s < 0:
                continue
            Ij = sl["I"]
            th = int(t_hi[s])
            for ki in range(len(sl["bo"])):
                k, bc, poff, rows = sl["bo"][ki]
                _, sc, _, _ = sl["so"][ki]
                r0 = k * P
                nrows = max(0, min(rows, th - r0))
                if nrows == 0:
                    continue
                iw = min(Ij, blur_np.shape[2])
                bimg[poff : poff + nrows, bc : bc + iw] = blur_np[
                    s, r0 : r0 + nrows, :iw
                ]
                simg[poff : poff + nrows, sc : sc + D] = seq_np[
                    s, r0 : r0 + nrows, :
                ]
        in_maps.append({"blur": bimg, "seq": simg})
    return in_maps


def _scatter_outputs(grid, layout, results):
    avged = np.zeros((B, T, D), np.float32)
    for c in range(N_CORES):
        oimg = results[c]["out"]
        for j, sl in enumerate(layout.slots):
            s = grid[j][c]
            if s < 0:
                continue
            Ij, oo = sl["I"], sl["oo"]
            avged[s, :Ij, :] = oimg[:, oo : oo + Ij].T
    return avged


# ---------------------------------------------------------------- entry

def _ramp(lens, Tn):
    l = np.asarray(lens).astype(np.float32)[:, None]
    pos = np.arange(Tn, dtype=np.float32)[None, :]
    return np.where(pos < l, (pos + 1.0) / np.maximum(l, 1.0), np.float32(0.0))


def _prepare(blur_np, seq_np):
    """Plan + (cached) compile + packed inputs. Returns None if all-zero."""
    t_hi, i_hi = _bounding_boxes(blur_np)
    plan = _plan(t_hi, i_hi)
    if plan is None:
        return None
    grid, template = plan
    layout = _Layout(template)
    key = tuple(template)
    if key not in _cache:
        _cache[key] = _build_program(layout)
    nc = _cache[key]
    in_maps = _pack_inputs(grid, layout, blur_np, seq_np, t_hi)
    return nc, grid, layout, in_maps


def kernel(seq, len_seq, SeqtoBlur, BlurMat, avged_len):
    seq_np = np.ascontiguousarray(np.asarray(seq), dtype=np.float32)
    blur_np = np.ascontiguousarray(np.asarray(BlurMat), dtype=np.float32)

    prep = _prepare(blur_np, seq_np)
    if prep is None:
        avged_seq = np.zeros((B, T, D), np.float32)
    else:
        nc, grid, layout, in_maps = prep
        res = run_bass_kernel_spmd(nc, in_maps, core_ids=list(range(N_CORES)))
        avged_seq = _scatter_outputs(grid, layout, res.results)

    R = _ramp(len_seq, T)
    avged_R = _ramp(avged_len, T)
    return (
        np.asarray(SeqtoBlur),
        avged_seq,
        R,
        avged_R,
        np.asarray(avged_len),
    )


# revision 21
# speedup vs baseline: 1.1565x; 1.1565x over previous
"""Trainium2 Bass kernel for BlurGenerationPair.

Reference computation (B=128, T=512, D=128):
    avged_seq[b] = BlurMat[b]^T @ seq[b]          # the only heavy compute
    R[b, t]      = (t+1)/len_seq[b]  for t < len_seq[b] else 0
    avged_R[b,t] = (t+1)/avged_len[b] for t < avged_len[b] else 0
    outputs: (SeqtoBlur, avged_seq, R, avged_R, avged_len)

SeqtoBlur and avged_len are identity pass-throughs and R/avged_R are tiny
host-side ramps, so the device work is the batched ragged matmul.

Strategy: BlurMat[b] is highly structured — all nonzeros live in a
[t_hi, i_hi] top-left rectangle (t_hi ~ len_seq[b], i_hi ~ avged_len[b]),
and ~20% of samples are entirely zero. The host computes each sample's
nonzero bounding box, load-balances the nonzero samples across the 8
cores (data parallel over batch), and packs each core's cropped blocks
directly in the SBUF image layout the matmuls want ([128 partitions x
free], contraction k-tiles side by side). Remainder k-tiles (< 128 rows)
from different samples are stacked vertically in shared column strips at
partition offsets {0, 32, 64} so the transferred rectangles carry almost
no dead bytes. The device program is a handful of large contiguous DMAs
per core plus the matmul pyramid (out.T = seq.T @ blur per sample, PSUM
accumulated over k-tiles). Per-slot shapes are static: the max over the
8 cores at each slot rank. Output rectangles are scattered back into a
zero-filled full-shape array on the host.
"""

import numpy as np

import concourse.bacc as bacc
import concourse.mybir as mybir
from concourse.tile import TileContext
from concourse.bass_utils import run_bass_kernel_spmd

B, T, D = 128, 512, 128
N_CORES = 8
P = 128

# pipelining granularity: chunk the strip sequence so each blur group
# transfer is at least this many bytes
GROUP_BYTES = 512 * 1024

_cache = {}


# ---------------------------------------------------------------- planning

def _bounding_boxes(blur):
    """Per-sample [t_hi, i_hi) nonzero bounding boxes of blur [B,T,T]."""
    nz_rows = np.any(blur != 0.0, axis=2)  # [B, T]
    nz_cols = np.any(blur != 0.0, axis=1)  # [B, T]
    t_hi = np.zeros(B, np.int64)
    i_hi = np.zeros(B, np.int64)
    for b in range(B):
        r = np.nonzero(nz_rows[b])[0]
        c = np.nonzero(nz_cols[b])[0]
        if r.size:
            t_hi[b] = r[-1] + 1
            i_hi[b] = c[-1] + 1
    return t_hi, i_hi


def _arrange_bin(members):
    """HW matmul partition-access rule: base 0 -> span <=128, base 32 ->
    span <=32, base 64 -> span <=64 (base 96 illegal). Place members
    (idx, rows) ascending by rows with a cursor rounded up to the
    smallest legal base. Returns [(idx, rows, offset)] or None."""
    span = {0: P, 32: 32, 64: 64}
    placed = []
    cur = 0
    for ix, r in sorted(members, key=lambda m: m[1]):
        off = None
        for base in (0, 32, 64):
            if base >= cur and r <= span[base] and base + r <= P:
                off = base
                break
        if off is None:
            return None
        placed.append((ix, r, off))
        cur = off + r
    return placed


def _bin_pack_rems(rems):
    """Greedy bin packing of remainder k-tiles (rows, width) into 128-row
    strips under the partition-base access rules.
    Returns list of bins: (width, [(item_idx, rows, offset)])."""
    order = sorted(range(len(rems)), key=lambda x: -rems[x][1])
    bins = []  # (wmax, [(idx, rows)])
    for ix in order:
        r, w = rems[ix]
        best = None
        for bi, (wmax, members) in enumerate(bins):
            if _arrange_bin(members + [(ix, r)]) is None:
                continue
            if any(abs(m[0] - ix) > 4 for m in members):
                continue  # keep members at nearby slot ranks (psum lifetime)
            used = sum(m[1] for m in members)
            c = max(0, w - wmax) * used + max(0, wmax - w) * r
            if best is None or c < best[0]:
                best = (c, bi)
        newc = (P - r) * w
        if best is not None and best[0] <= newc:
            bi = best[1]
            wmax, members = bins[bi]
            bins[bi] = (max(wmax, w), members + [(ix, r)])
        else:
            bins.append((w, [(ix, r)]))
    return [(wmax, _arrange_bin(members)) for wmax, members in bins]


def _grid_cost(grid, t32f, i4f):
    """Proxy for per-core image bytes of an assignment (incl remainder
    stacking)."""
    tot = 0
    rems = []
    for row in grid:
        Ts = [t32f[s] for s in row if s >= 0]
        Is = [i4f[s] for s in row if s >= 0]
        if not Ts:
            continue
        Tj, Ij = max(Ts), max(Is)
        Kf, r = Tj // P, Tj % P
        tot += Kf * Ij * P + Kf * P * P + Ij * P
        if r:
            rems.append((r, Ij))
    for wmax, _members in _bin_pack_rems(rems):
        tot += P * wmax + P * P
    return tot * 4


def _plan(t_hi, i_hi):
    """Assign nonzero samples to (core, slot) cells and build the static
    per-slot shape template (max over cores at each slot rank).
    t is at 32-row grain (partition-offset stacking granularity),
    i at 4-col grain."""
    nz = np.nonzero(t_hi)[0]
    if nz.size == 0:
        return None
    t32 = np.minimum((t_hi[nz] + 31) // 32 * 32, T)
    i4 = np.minimum((i_hi[nz] + 3) // 4 * 4, T)
    cost = t32 * (i4 + P) + i4 * P
    order = np.argsort(-cost, kind="stable")
    S = int(np.ceil(nz.size / N_CORES))
    grid = []
    k = 0
    for j in range(S):
        row = []
        for c in range(N_CORES):
            row.append(nz[order[k]] if k < nz.size else -1)
            k += 1
        if j % 2 == 1:
            row = row[::-1]  # serpentine for per-core cost balance
        grid.append(row)

    t32f = np.zeros(B, np.int64)
    i4f = np.zeros(B, np.int64)
    t32f[nz] = t32
    i4f[nz] = i4

    def tmpl(row):
        Ts = [t32f[s] for s in row if s >= 0]
        Is = [i4f[s] for s in row if s >= 0]
        if not Ts:
            return 0, 0
        return max(Ts), max(Is)

    # local-swap refinement: any pair of slot rows within a core
    for _ in range(6):
        improved = False
        for j1 in range(S):
            for j2 in range(j1 + 1, S):
                for c in range(N_CORES):
                    a, b_ = grid[j1][c], grid[j2][c]
                    if a < 0 or b_ < 0:
                        continue
                    cur = _grid_cost(grid, t32f, i4f)
                    grid[j1][c], grid[j2][c] = b_, a
                    new = _grid_cost(grid, t32f, i4f)
                    if new < cur:
                        improved = True
                    else:
                        grid[j1][c], grid[j2][c] = a, b_
        if not improved:
            break

    template = [tmpl(row) for row in grid]
    keep = [j for j, t in enumerate(template) if t[0] > 0 and t[1] > 0]
    return [grid[j] for j in keep], [template[j] for j in keep]


class _Layout:
    """Static geometry shared by program builder, packer and scatterer.

    Blur/seq images are sequences of column strips:
      - full strip: one slot's k-tile, 128 rows of data
      - bin strip: stacked remainder k-tiles of 1-3 slots at partition
        offsets {0, 32, 64}
    Strips are positioned in slot order (bins at their first member), so a
    slot's dependencies are always in its own or earlier DMA groups.
    """

    def __init__(self, template):
        self.template = template
        S = len(template)
        self.slots = []
        for Tj, Ij in template:
            self.slots.append(
                dict(T=Tj, I=Ij, Kf=Tj // P, r=Tj % P, bo=[], so=[], oo=0)
            )

        # pass 1: full strips in slot order; remainder k-tiles bin-packed
        # into shared strips inserted after their LAST member's full strips.
        # strip: dict(w=blur width, members=[(slot, kind, k, poff, rows)])
        rems = []  # (slot, rows, width), in slot order
        for j, sl in enumerate(self.slots):
            if sl["r"]:
                rems.append((j, sl["r"], sl["I"]))
        bin_after = {}  # slot -> list of bin strip dicts to insert after it
        for wmax, placed in _bin_pack_rems([(r, w) for _j, r, w in rems]):
            members = []
            for ix, r, off in placed:
                j = rems[ix][0]
                members.append((j, "rem", self.slots[j]["Kf"], off, r))
            last = max(m[0] for m in members)
            bin_after.setdefault(last, []).append(
                dict(w=wmax, members=members, bin=True)
            )
        self.strips = []
        for j, sl in enumerate(self.slots):
            Ij = sl["I"]
            for k in range(sl["Kf"]):
                self.strips.append(
                    dict(w=Ij, members=[(j, "full", k, 0, P)], bin=False)
                )
            for st in bin_after.get(j, []):
                self.strips.append(st)

        # pass 2: column offsets; per-slot k-tile -> (strip col, poff, rows)
        bc = sc = oc = 0
        for st in self.strips:
            st["bc"] = bc
            st["sc"] = sc
            for (j, kind, k, poff, rows) in st["members"]:
                self.slots[j]["bo"].append((k, bc, poff, rows))
                self.slots[j]["so"].append((k, sc, poff, rows))
            bc += st["w"]
            sc += D
        for sl in self.slots:
            sl["bo"].sort()
            sl["so"].sort()
            sl["oo"] = oc
            oc += sl["I"]
        self.blur_cols = bc
        self.seq_cols = sc
        self.out_cols = oc

        # groups: consecutive strips, >= GROUP_BYTES of blur each
        self.groups = []  # list of (strip_lo, strip_hi)
        lo = 0
        acc = 0
        for si, st in enumerate(self.strips):
            acc += st["w"] * P * 4
            if acc >= GROUP_BYTES:
                self.groups.append((lo, si + 1))
                lo = si + 1
                acc = 0
        if lo < len(self.strips):
            self.groups.append((lo, len(self.strips)))
        # strip index -> group index
        self.strip_group = {}
        for g, (a, b_) in enumerate(self.groups):
            for si in range(a, b_):
                self.strip_group[si] = g
        # slot -> last group it depends on (for compute emission order)
        slot_last_group = [0] * S
        for si, st in enumerate(self.strips):
            for (j, *_rest) in st["members"]:
                slot_last_group[j] = max(slot_last_group[j], self.strip_group[si])
        self.group_slots = [[] for _ in self.groups]
        for j in range(S):
            self.group_slots[slot_last_group[j]].append(j)


# ---------------------------------------------------------------- program

def _build_program(layout, reps=None):
    """reps: if set, wrap the body in a hardware loop executing it `reps`
    times — used only for steady-state benchmarking (amortizes the ~90ms
    axon dispatch overhead out of wall-clock measurements)."""
    import contextlib

    nc = bacc.Bacc("TRN2")
    f32 = mybir.dt.float32

    blur_t = nc.dram_tensor("blur", [P, layout.blur_cols], f32, kind="ExternalInput")
    seq_t = nc.dram_tensor("seq", [P, layout.seq_cols], f32, kind="ExternalInput")
    out_t = nc.dram_tensor("out", [P, layout.out_cols], f32, kind="ExternalOutput")

    with TileContext(nc) as tc:
        with (
            tc.tile_pool(name="sb", bufs=1) as sb,
            tc.tile_pool(name="psum", bufs=8, space="PSUM") as psum_p,
            tc.For_i(0, reps, 1) if reps else contextlib.nullcontext(),
        ):
            qbytes = [0, 0]

            def q_engine(nbytes):
                qi = 0 if qbytes[0] <= qbytes[1] else 1
                qbytes[qi] += nbytes
                return nc.sync if qi == 0 else nc.scalar

            # SBUF residency: keep all groups resident when they fit in the
            # SBUF budget; otherwise ring-buffer the group tags (Tile then
            # serializes a reused slot's load behind its prior consumers).
            ngroups = len(layout.groups)
            total_cols = layout.blur_cols + layout.seq_cols + min(
                len(layout.slots), 8
            ) * 512
            budget_cols = 160 * 1024 // 4  # ~160KB per partition
            ring = ngroups
            if total_cols > budget_cols:
                gw = []
                for a, b_ in layout.groups:
                    bc1 = layout.blur_cols if b_ == len(layout.strips) else layout.strips[b_]["bc"]
                    gw.append(bc1 - layout.strips[a]["bc"] + (b_ - a) * D)
                wmax = max(gw)
                ring = max(3, (budget_cols - 8 * 512) // max(wmax, 1))

            # per-group SBUF tiles + group column base
            blur_tiles = []
            seq_tiles = []
            gbase = []
            for g, (a, b_) in enumerate(layout.groups):
                bc0 = layout.strips[a]["bc"]
                sc0 = layout.strips[a]["sc"]
                bc1 = layout.blur_cols if b_ == len(layout.strips) else layout.strips[b_]["bc"]
                sc1 = layout.seq_cols if b_ == len(layout.strips) else layout.strips[b_]["sc"]
                gbase.append((bc0, sc0))
                bt = sb.tile([P, bc1 - bc0], f32, tag=f"blur{g % ring}")
                q_engine((bc1 - bc0) * P * 4).dma_start(out=bt[:], in_=blur_t[:, bc0:bc1])
                st = sb.tile([P, sc1 - sc0], f32, tag=f"seq{g % ring}")
                q_engine((sc1 - sc0) * P * 4).dma_start(out=st[:], in_=seq_t[:, sc0:sc1])
                blur_tiles.append(bt)
                seq_tiles.append(st)

                # compute for slots whose last dependency is group g
                for j in layout.group_slots[g]:
                    sl = layout.slots[j]
                    Ij, oo = sl["I"], sl["oo"]
                    nk = len(sl["bo"])
                    ps = psum_p.tile([P, 512], f32, tag="ps")
                    for ki, (k, bc, poff, rows) in enumerate(sl["bo"]):
                        _, sc, _, _ = sl["so"][ki]
                        sg = layout.strip_group[
                            _strip_at(layout, bc)
                        ]
                        bg0, sg0 = gbase[sg]
                        nc.tensor.matmul(
                            ps[:, :Ij],
                            lhsT=seq_tiles[sg][poff : poff + rows, sc - sg0 : sc - sg0 + D],
                            rhs=blur_tiles[sg][poff : poff + rows, bc - bg0 : bc - bg0 + Ij],
                            start=(ki == 0),
                            stop=(ki == nk - 1),
                        )
                    og = sb.tile([P, Ij], f32, tag=f"out{j % 8}")
                    nc.vector.tensor_copy(og[:], ps[:, :Ij])
                    q_engine(Ij * P * 4).dma_start(
                        out=out_t[:, oo : oo + Ij], in_=og[:]
                    )
    nc.finalize()
    return nc


def _strip_at(layout, bc):
    # strips are ordered by bc; binary search
    lo, hi = 0, len(layout.strips)
    while lo + 1 < hi:
        mid = (lo + hi) // 2
        if layout.strips[mid]["bc"] <= bc:
            lo = mid
        else:
            hi = mid
    return lo


# ---------------------------------------------------------------- packing

def _pack_inputs(grid, layout, blur_np, seq_np, t_hi):
    """Build per-core SBUF-image blur/seq buffers [128, cols]."""
    in_maps = []
    for c in range(N_CORES):
        bimg = np.zeros((P, layout.blur_cols), np.float32)
        simg = np.zeros((P, layout.seq_cols), np.float32)
        for j, sl in enumerate(layout.slots):
            s = grid[j][c]
            if s < 0:
                continue
            Ij = sl["I"]
            th = int(t_hi[s])
            for ki in range(len(sl["bo"])):
                k, bc, poff, rows = sl["bo"][ki]
                _, sc, _, _ = sl["so"][ki]
                r0 = k * P
                nrows = max(0, min(rows, th - r0))
                if nrows == 0:
                    continue
                iw = min(Ij, blur_np.shape[2])
                bimg[poff : poff + nrows, bc : bc + iw] = blur_np[
                    s, r0 : r0 + nrows, :iw
                ]
                simg[poff : poff + nrows, sc : sc + D] = seq_np[
                    s, r0 : r0 + nrows, :
                ]
        in_maps.append({"blur": bimg, "seq": simg})
    return in_maps


def _scatter_outputs(grid, layout, results):
    avged = np.zeros((B, T, D), np.float32)
    for c in range(N_CORES):
        oimg = results[c]["out"]
        for j, sl in enumerate(layout.slots):
            s = grid[j][c]
            if s < 0:
                continue
            Ij, oo = sl["I"], sl["oo"]
            avged[s, :Ij, :] = oimg[:, oo : oo + Ij].T
    return avged


# ---------------------------------------------------------------- entry

def _ramp(lens, Tn):
    l = np.asarray(lens).astype(np.float32)[:, None]
    pos = np.arange(Tn, dtype=np.float32)[None, :]
    return np.where(pos < l, (pos + 1.0) / np.maximum(l, 1.0), np.float32(0.0))


def _prepare(blur_np, seq_np):
    """Plan + (cached) compile + packed inputs. Returns None if all-zero."""
    t_hi, i_hi = _bounding_boxes(blur_np)
    plan = _plan(t_hi, i_hi)
    if plan is None:
        return None
    grid, template = plan
    layout = _Layout(template)
    key = tuple(template)
    if key not in _cache:
        _cache[key] = _build_program(layout)
    nc = _cache[key]
    in_maps = _pack_inputs(grid, layout, blur_np, seq_np, t_hi)
    return nc, grid, layout, in_maps


def kernel(seq, len_seq, SeqtoBlur, BlurMat, avged_len):
    seq_np = np.ascontiguousarray(np.asarray(seq), dtype=np.float32)
    blur_np = np.ascontiguousarray(np.asarray(BlurMat), dtype=np.float32)

    prep = _prepare(blur_np, seq_np)
    if prep is None:
        avged_seq = np.zeros((B, T, D), np.float32)
    else:
        nc, grid, layout, in_maps = prep
        core_ids = list(range(N_CORES))
        try:
            res = run_bass_kernel_spmd(nc, in_maps, core_ids=core_ids)
        except ModuleNotFoundError:
            # BASS_TRACE=1 under axon needs a profiling hook module that is
            # not present in every container; retry with tracing disabled.
            import os

            os.environ["BASS_NEVER_TRACE"] = "1"
            try:
                res = run_bass_kernel_spmd(nc, in_maps, core_ids=core_ids)
            finally:
                os.environ.pop("BASS_NEVER_TRACE", None)
        avged_seq = _scatter_outputs(grid, layout, res.results)

    R = _ramp(len_seq, T)
    avged_R = _ramp(avged_len, T)
    return (
        np.asarray(SeqtoBlur),
        avged_seq,
        R,
        avged_R,
        np.asarray(avged_len),
    )
